# revision 14
# baseline (speedup 1.0000x reference)
# Trainium2 Bass kernel for nn_FPN_AAR (dense_cnn): 5-branch deformable-sampled
# 3x3 conv + SKConv attention fusion, sharded over 8 NeuronCores by output rows.
#
#   - Each core computes an 18-row slab of the 142-row output for all 5 angle
#     branches (uniform geometry; core 7 overhangs and the host keeps only its
#     valid 16 rows).
#   - Offset sampling runs on-chip with constant weights: clipped sample
#     indices always land in zero padding, so a zero-margin input slab makes
#     every border case identical to the interior.
#   - Convs run as 18 accumulating fp32r matmuls per PSUM bank (9 taps x 2
#     cin chunks), N=426 (3 output rows x 142).
#   - BN+ReLU on ScalarE with per-channel bias (gamma' folded into attention
#     weights); spatial partial sums come free via accum_out.
#   - Cross-core: AllGather of per-group partial sums; every core redundantly
#     computes the tiny fc1/fc2/softmax and applies attention to its slab.
import sys
import math

sys.path.insert(0, "/opt/trn_rl_repo")

import numpy as np

KS = 3
S2 = 2**0.5
ANGLES = (0, 45, 90, 135, 180)
BN_EPS = 1e-5
NCORES = 8
B, C, H, W = 2, 256, 48, 48
HOUT = 142
NI = 7          # sample rows per core
NROWS = 18      # output rows per core slab
NG = 6          # row groups of 3
NTILE = 3 * HOUT  # 426
I0S = [6 * k for k in range(7)] + [42]


def _angle_offsets(angle):
    n = angle // 45
    if n == 0:
        ox = [0.0] * 9
        oy = [0.0] * 9
    elif n == 1:
        ox = [1 - S2, 1 - S2 * 0.5, 1, -S2 * 0.5, 0, S2 * 0.5, -1, S2 * 0.5 - 1, S2 - 1]
        oy = [1, S2 * 0.5, S2 - 1, 1 - S2 * 0.5, 0, S2 * 0.5 - 1, 1 - S2, -S2 * 0.5, -1]
    elif n == 2:
        ox = [0, 1, 2, -1, 0, 1, -2, -1, 0]
        oy = [2, 1, 0, 1, 0, -1, 0, -1, -2]
    elif n == 3:
        ox = [1, 1 + S2 * 0.5, 1 + S2, -S2 * 0.5, 0, S2 * 0.5, -1 - S2, -1 - S2 * 0.5, -1]
        oy = [1 + S2, S2 * 0.5, -1, 1 + S2 * 0.5, 0, -1 - S2 * 0.5, 1, -S2 * 0.5, 1 + S2]
    else:
        ox = [2, 2, 2, 0, 0, 0, -2, -2, -2]
        oy = [2, 0, -2, 2, 0, -2, 2, 0, -2]
    return ox, oy


def _angle_terms(angle):
    """Per kernel point p=(r,s): sampling as up to 2x2 separable terms.

    xo[c, 3I+r, 3j+s] = sum_u sum_v wx_u*wy_v * xs[c, I+dx_u+4, j+dy_v+4]
    where the input slab xs carries zero margins so clipped samples read 0,
    matching the reference's clamp-into-padding semantics exactly.
    """
    ox, oy = _angle_offsets(angle)
    pn = [-1.0, 0.0, 1.0]
    out = {}
    for p in range(9):
        cx = 1.0 + pn[p // 3] + float(ox[p])
        cy = 1.0 + pn[p % 3] + float(oy[p])
        fx, fy = math.floor(cx), math.floor(cy)
        ax, ay = cx - fx, cy - fy
        xterms = [(int(d), w) for d, w in [(fx, 1 - ax), (fx + 1, ax)] if w != 0.0]
        yterms = [(int(d), w) for d, w in [(fy, 1 - ay), (fy + 1, ay)] if w != 0.0]
        out[p] = (xterms, yterms)
    return out


_BUILD_CACHE = {}


def _build_program():
    if "nc" in _BUILD_CACHE:
        return _BUILD_CACHE["nc"]

    import concourse.bacc as bacc
    import concourse.tile as tile
    import concourse.mybir as mybir

    f32 = mybir.dt.float32
    f32r = mybir.dt.float32r
    AF = mybir.ActivationFunctionType
    ALU = mybir.AluOpType
    AX = mybir.AxisListType

    nc = bacc.Bacc("TRN2", target_bir_lowering=False, debug=False,
                   num_devices=NCORES)

    xs_d = nc.dram_tensor("xs", [128, 4, 16, 57], f32, kind="ExternalInput")
    wt_d = nc.dram_tensor("wt", [128, 2, 2, 9, 128], f32r, kind="ExternalInput")
    bias_d = nc.dram_tensor("bias_act", [128, 2, 4], f32, kind="ExternalInput")
    gpr_d = nc.dram_tensor("gpr", [128, 2, 4], f32, kind="ExternalInput")
    w1t_d = nc.dram_tensor("w1t", [128, 2, 32], f32, kind="ExternalInput")
    b1_d = nc.dram_tensor("b1", [32, 1], f32, kind="ExternalInput")
    w2t_d = nc.dram_tensor("w2t", [32, 4, 2, 128], f32, kind="ExternalInput")
    b2t_d = nc.dram_tensor("b2t", [128, 2, 4], f32, kind="ExternalInput")
    out_d = nc.dram_tensor("out", [128, 4, NROWS, HOUT], f32, kind="ExternalOutput")

    terms = {a: _angle_terms(a) for a in ANGLES}
    branch_of = {45: 0, 90: 1, 135: 2, 180: 3}
    angle_order = [45, 135, 90, 180, 0]

    with tile.TileContext(nc) as tc:
        with tc.tile_pool(name="persist", bufs=1) as pp, \
             tc.tile_pool(name="xof", bufs=1) as xofp, \
             tc.tile_pool(name="xor", bufs=3) as xorp, \
             tc.tile_pool(name="work", bufs=3) as wp, \
             tc.tile_pool(name="apply", bufs=2) as app, \
             tc.tile_pool(name="y0p", bufs=14) as y0p, \
             tc.tile_pool(name="psum", bufs=8, space="PSUM") as psp, \
             tc.tile_pool(name="dram", bufs=1, space="DRAM") as dp:

            # ---- persistent loads ----
            xs_t = []
            for _bc in range(4):
                xst = pp.tile([128, 16, 57], f32, tag=f"xs{_bc}",
                              name=f"xs{_bc}")
                nc.sync.dma_start(xst[:], xs_d.ap()[:, _bc])
                xs_t.append(xst)
            wtr_t = []
            for _oc in range(2):
                wto = pp.tile([128, 2, 9, 128], f32r, tag=f"wtr{_oc}",
                              name=f"wtr{_oc}")
                nc.sync.dma_start(wto[:], wt_d.ap()[:, :, _oc])
                wtr_t.append(wto)
            bias_sb = pp.tile([128, 2, 4], f32, tag="bias")
            nc.sync.dma_start(bias_sb[:], bias_d.ap()[:])
            gpr_sb = pp.tile([128, 2, 4], f32, tag="gpr")
            nc.sync.dma_start(gpr_sb[:], gpr_d.ap()[:])
            w1t_sb = pp.tile([128, 2, 32], f32, tag="w1t")
            nc.sync.dma_start(w1t_sb[:], w1t_d.ap()[:])
            b1_sb = pp.tile([32, 1], f32, tag="b1")
            nc.sync.dma_start(b1_sb[:], b1_d.ap()[:])
            w2t_sb = pp.tile([32, 4, 2, 128], f32, tag="w2t")
            nc.sync.dma_start(w2t_sb[:], w2t_d.ap()[:])
            b2t_sb = pp.tile([128, 2, 4], f32, tag="b2t")
            nc.sync.dma_start(b2t_sb[:], b2t_d.ap()[:])

            # PE warm-up: dependency-free matmuls on zeroed tiles run while
            # the input DMAs land, releasing the HAM clock gate before the
            # first real conv matmul.
            wz = pp.tile([128, 512], mybir.dt.bfloat16, tag="warmz")
            nc.vector.memset(wz[:], 0.0)
            wps = psp.tile([128, 512], f32, tag="ps", name="warm_ps")
            for _wi in range(40):
                nc.tensor.matmul(wps[:], wz[:, 0:128], wz[:],
                                 start=True, stop=True)

            # acc layout: [m(4), b(2), oc(2), acc(8)]
            # acc 0..4 = row-group sums g0..g4, acc 5..7 = rows 15,16,17
            acc_sb = pp.tile([128, 4, 2, 2, 8], f32, tag="acc")

            feas_dram = dp.tile([4, 2, 2, NG, 128, NTILE], mybir.dt.bfloat16,
                                tag="feas_d")
            ag_in = dp.tile([128, 128], f32, tag="ag_in")
            ag_out = dp.tile([128 * NCORES, 128], f32, tag="ag_out")

            attp = pp.tile([128, 2, 2, 4], f32, tag="attp")

            # ---- helpers ----
            def emit_sample(a):
                xo_r = {}
                for bb in range(2):
                    for cc in range(2):
                        xsl = xs_t[bb * 2 + cc][:]  # [128, 16, 57]
                        xof = xofp.tile([128, 21, 144], f32, tag="xof",
                                        name=f"xof_{a}_{bb}_{cc}")
                        for p in range(9):
                            r, s = p // 3, p % 3
                            xterms, yterms = terms[a][p]
                            dst = xof[:, r::3, s::3]    # [128, 7, 48]
                            if len(xterms) == 1 and len(yterms) == 1:
                                dx, wx = xterms[0]
                                dy, wy = yterms[0]
                                src = xsl[:, 4 + dx:4 + dx + 7, 4 + dy:4 + dy + 48]
                                w = wx * wy
                                if w == 1.0:
                                    nc.gpsimd.tensor_copy(dst, src)
                                else:
                                    nc.gpsimd.tensor_scalar_mul(dst, src, float(w))
                            else:
                                if len(xterms) == 1:
                                    dx, wx = xterms[0]
                                    At = xsl[:, 4 + dx:4 + dx + 7, :]
                                    ascale = wx
                                else:
                                    (dx0, wx0), (dx1, wx1) = xterms
                                    Atile = wp.tile([128, 7, 57], f32, tag="stepA", bufs=2,
                                                    name=f"sa_{a}_{bb}_{cc}_{p}")
                                    t2 = wp.tile([128, 7, 57], f32, tag="stepA2", bufs=2,
                                                 name=f"sa2_{a}_{bb}_{cc}_{p}")
                                    nc.vector.tensor_scalar_mul(
                                        Atile[:], xsl[:, 4 + dx0:4 + dx0 + 7, :],
                                        float(wx0))
                                    nc.vector.tensor_scalar_mul(
                                        t2[:], xsl[:, 4 + dx1:4 + dx1 + 7, :],
                                        float(wx1))
                                    nc.vector.tensor_add(Atile[:], Atile[:], t2[:])
                                    At = Atile[:]
                                    ascale = 1.0
                                if len(yterms) == 1:
                                    dy, wy = yterms[0]
                                    w = ascale * wy
                                    src = At[:, :, 4 + dy:4 + dy + 48]
                                    if w == 1.0:
                                        nc.scalar.copy(dst, src)
                                    else:
                                        nc.scalar.mul(dst, src, float(w))
                                else:
                                    (dy0, wy0), (dy1, wy1) = yterms
                                    tb = wp.tile([128, 7, 48], f32, tag="stepB", bufs=2,
                                                 name=f"sb_{a}_{bb}_{cc}_{p}")
                                    nc.vector.tensor_scalar_mul(
                                        dst, At[:, :, 4 + dy0:4 + dy0 + 48],
                                        float(ascale * wy0))
                                    nc.vector.tensor_scalar_mul(
                                        tb[:], At[:, :, 4 + dy1:4 + dy1 + 48],
                                        float(ascale * wy1))
                                    nc.vector.tensor_add(dst, dst, tb[:])
                        xr = xorp.tile([128, 21, 144], f32r, tag=f"xor{cc}",
                                       name=f"xor_{a}_{bb}_{cc}")
                        nc.vector.tensor_copy(xr[:], xof[:])
                        xo_r[(bb, cc)] = xr
                return xo_r

            def emit_conv(a, bb, oc, xo_r):
                banks = [psp.tile([128, 3, HOUT], f32, tag="ps",
                                  name=f"ps_{a}_{bb}_{oc}_{g}")
                         for g in range(NG)]
                for tap in range(9):
                    di, dj = tap // 3, tap % 3
                    for cc in range(2):
                        lhsT = wtr_t[oc][:, cc, tap, :]
                        xr = xo_r[(bb, cc)]
                        first = (tap == 0 and cc == 0)
                        last = (tap == 8 and cc == 1)
                        for g in range(NG):
                            rhs = xr[:, 3 * g + di:3 * g + di + 3, dj:dj + HOUT]
                            nc.tensor.matmul(banks[g][:], lhsT, rhs,
                                             start=first, stop=last)
                return banks

            def emit_drain_branch(a, bb, oc, banks):
                m = branch_of[a]
                for g in range(NG):
                    ft = wp.tile([128, 3, HOUT], mybir.dt.bfloat16, tag="feas_t",
                                 name=f"ft_{a}_{bb}_{oc}_{g}")
                    if g < 5:
                        nc.scalar.activation(
                            ft[:], banks[g][:], AF.Relu,
                            bias=bias_sb[:, oc, m:m + 1],
                            scale=gpr_sb[:, oc, m:m + 1],
                            accum_out=acc_sb[:, m, bb, oc, g:g + 1])
                    else:
                        for r in range(3):
                            nc.scalar.activation(
                                ft[:, r, :], banks[g][:, r, :], AF.Relu,
                                bias=bias_sb[:, oc, m:m + 1],
                                scale=gpr_sb[:, oc, m:m + 1],
                                accum_out=acc_sb[:, m, bb, oc, 5 + r:6 + r])
                    nc.sync.dma_start(feas_dram[m, bb, oc, g, :, :],
                                      ft[:].rearrange("p a b -> p (a b)"))

            def emit_drain_y0(bb, oc, banks):
                tiles = []
                for g in range(NG):
                    yt = y0p.tile([128, 3, HOUT], f32, tag="y0sb",
                                  name=f"y0_{bb}_{oc}_{g}")
                    nc.scalar.copy(yt[:], banks[g][:])
                    tiles.append(yt)
                return tiles

            def emit_collective_reduce():
                nc.sync.dma_start(
                    ag_in[:, :],
                    acc_sb[:].rearrange("p a b c d -> p (a b c d)"))
                nc.gpsimd.collective_compute(
                    "AllGather", ALU.bypass,
                    replica_groups=[list(range(NCORES))],
                    ins=[ag_in.opt()], outs=[ag_out.opt()])
                ag_sb = pp.tile([128, NCORES, 16, 8], f32, tag="ag_sb")
                nc.sync.dma_start(
                    ag_sb[:],
                    ag_out[:].rearrange("(s p) (mbo a) -> p s mbo a",
                                        p=128, a=8))
                r1 = pp.tile([128, NCORES, 16], f32, tag="r1")
                nc.vector.reduce_sum(r1[:], ag_sb[:, :, :, 0:6], axis=AX.X)
                r2 = pp.tile([128, NCORES - 1, 16], f32, tag="r2")
                nc.vector.reduce_sum(r2[:], ag_sb[:, 0:7, :, 6:8], axis=AX.X)
                fm = pp.tile([128, 16], f32, tag="fm")
                fm2 = pp.tile([128, 16], f32, tag="fm2")
                nc.vector.reduce_sum(fm[:], r1[:].transpose([0, 2, 1]),
                                     axis=AX.X)
                nc.vector.reduce_sum(fm2[:], r2[:].transpose([0, 2, 1]),
                                     axis=AX.X)
                nc.vector.tensor_add(fm[:], fm[:], fm2[:])
                # fm layout [m, b, oc]; feas sums are already BN'd
                fmv = fm[:].rearrange("p (m b o) -> p m b o", m=4, b=2)
                fs = pp.tile([128, 2, 2], f32, tag="fs")
                nc.vector.reduce_sum(fs[:], fmv.transpose([0, 2, 3, 1]),
                                     axis=AX.X)
                nc.vector.tensor_scalar_mul(fs[:], fs[:], 1.0 / (HOUT * HOUT))
                return fs

            def emit_fc_softmax(fs):
                pz = psp.tile([32, 2], f32, tag="ps", name="pz_fc1")
                for cc in range(2):
                    nc.tensor.matmul(pz[:], w1t_sb[:, cc, :], fs[:, :, cc],
                                     start=(cc == 0), stop=(cc == 1))
                zt = pp.tile([32, 2], f32, tag="zt")
                nc.scalar.add(zt[:], pz[:], b1_sb[:, 0:1])
                logit = pp.tile([128, 2, 2, 4], f32, tag="logit")
                for m in range(4):
                    for oc in range(2):
                        p2 = psp.tile([128, 2], f32, tag="ps",
                                      name=f"p2_fc2_{m}_{oc}")
                        nc.tensor.matmul(p2[:], w2t_sb[:, m, oc, :], zt[:],
                                         start=True, stop=True)
                        nc.scalar.add(logit[:, oc, :, m], p2[:],
                                      b2t_sb[:, oc, m:m + 1])
                rmax = pp.tile([128, 2, 2, 1], f32, tag="rmax")
                nc.vector.tensor_reduce(rmax[:], logit[:], AX.X, ALU.max)
                nc.vector.tensor_tensor(
                    logit[:], logit[:],
                    rmax[:].broadcast_to([128, 2, 2, 4]), ALU.subtract)
                elog = pp.tile([128, 2, 2, 4], f32, tag="elog")
                nc.scalar.activation(elog[:], logit[:], AF.Exp)
                ssum = pp.tile([128, 2, 2, 1], f32, tag="ssum")
                nc.vector.reduce_sum(ssum[:], elog[:], axis=AX.X)
                rinv = pp.tile([128, 2, 2, 1], f32, tag="rinv")
                nc.vector.reciprocal(rinv[:], ssum[:])
                nc.vector.tensor_tensor(
                    attp[:], elog[:],
                    rinv[:].broadcast_to([128, 2, 2, 4]), ALU.mult)

            def emit_apply(bb, oc, y0sb):
                for g in range(NG):
                    fts = []
                    for m in range(4):
                        ft = app.tile([128, NTILE], mybir.dt.bfloat16,
                                      tag=f"ld{m}", bufs=5,
                                      name=f"ld_{bb}_{oc}_{g}_{m}")
                        nc.sync.dma_start(ft[:], feas_dram[m, bb, oc, g, :, :])
                        fts.append(ft)
                    acc = app.tile([128, NTILE], f32, tag="acc_t",
                                   name=f"acc_{bb}_{oc}_{g}")
                    tmp = app.tile([128, NTILE], f32, tag="tmp_t", bufs=1,
                                   name=f"tmp_{bb}_{oc}_{g}")
                    t1 = app.tile([128, NTILE], f32, tag="t1_t", bufs=1,
                                  name=f"t1_{bb}_{oc}_{g}")
                    t2 = app.tile([128, NTILE], f32, tag="t2_t", bufs=1,
                                  name=f"t2_{bb}_{oc}_{g}")
                    # scale the four branches on three different engines
                    nc.vector.tensor_scalar_mul(
                        acc[:], fts[0][:], attp[:, oc, bb, 0:1])
                    nc.scalar.mul(t1[:], fts[1][:], attp[:, oc, bb, 1:2])
                    nc.gpsimd.tensor_scalar_mul(
                        t2[:], fts[2][:], attp[:, oc, bb, 2:3])
                    nc.vector.tensor_scalar_mul(
                        tmp[:], fts[3][:], attp[:, oc, bb, 3:4])
                    nc.vector.tensor_add(acc[:], acc[:], t1[:])
                    nc.gpsimd.tensor_add(
                        t2[:], t2[:],
                        y0sb[g][:].rearrange("p a b -> p (a b)"))
                    nc.vector.tensor_add(tmp[:], tmp[:], t2[:])
                    nc.vector.tensor_add(acc[:], acc[:], tmp[:])
                    nc.sync.dma_start(
                        out_d.ap()[:, bb * 2 + oc, 3 * g:3 * g + 3, :],
                        acc[:].rearrange("p (a b) -> p a b", a=3))

            # ---- main schedule ----
            # BN branches first; collective overlaps angle-0 convs; fc matmuls
            # sit after two angle-0 phases so the in-order PE queue never
            # stalls on the collective; applies interleave with the remaining
            # angle-0 phases.
            for a in [90, 45, 135, 180]:
                xo_r = emit_sample(a)
                for bb in range(2):
                    for oc in range(2):
                        banks = emit_conv(a, bb, oc, xo_r)
                        emit_drain_branch(a, bb, oc, banks)

            fs = emit_collective_reduce()

            xo0 = emit_sample(0)
            y0sb = {}
            y0sb[(0, 0)] = emit_drain_y0(0, 0, emit_conv(0, 0, 0, xo0))
            emit_fc_softmax(fs)
            y0sb[(0, 1)] = emit_drain_y0(0, 1, emit_conv(0, 0, 1, xo0))
            emit_apply(0, 0, y0sb[(0, 0)])
            y0sb[(1, 0)] = emit_drain_y0(1, 0, emit_conv(0, 1, 0, xo0))
            emit_apply(0, 1, y0sb[(0, 1)])
            y0sb[(1, 1)] = emit_drain_y0(1, 1, emit_conv(0, 1, 1, xo0))
            emit_apply(1, 0, y0sb[(1, 0)])
            emit_apply(1, 1, y0sb[(1, 1)])

    nc.compile()
    _BUILD_CACHE["nc"] = nc
    return nc


def _host_prep(x, conv_w, bn_gamma, bn_beta, bn_mean, bn_var, fc1_w, fc1_b,
               fc2_w, fc2_b):
    x = np.asarray(x, np.float32)
    conv_w = np.asarray(conv_w, np.float32)
    x_ext = np.zeros((B, C, 60, 57), np.float32)
    x_ext[:, :, 5:53, 5:53] = x

    gprime = (np.asarray(bn_gamma) / np.sqrt(np.asarray(bn_var) + BN_EPS)) \
        .astype(np.float32)
    bprime = (np.asarray(bn_beta) - np.asarray(bn_mean) * gprime) \
        .astype(np.float32)
    bias_act = bprime                                    # [4, 256]

    # wt[ci, cc, oc, tap, co] = conv_w[oc*128+co, cc*128+ci, di, dj]
    w6 = conv_w.reshape(2, 128, 2, 128, 9)               # [oc, co, cc, ci, tap]
    wt = np.ascontiguousarray(w6.transpose(3, 2, 0, 4, 1))

    bias_t = np.ascontiguousarray(
        bias_act.reshape(4, 2, 128).transpose(2, 1, 0))  # [co, oc, m]
    gpr_t = np.ascontiguousarray(
        gprime.reshape(4, 2, 128).transpose(2, 1, 0))
    w1t = np.ascontiguousarray(
        np.asarray(fc1_w, np.float32).T.reshape(2, 128, 32).transpose(1, 0, 2))
    b1 = np.asarray(fc1_b, np.float32).reshape(32, 1).copy()
    w2t = np.ascontiguousarray(
        np.asarray(fc2_w, np.float32).reshape(4, 2, 128, 32)
        .transpose(3, 0, 1, 2))
    b2t = np.ascontiguousarray(
        np.asarray(fc2_b, np.float32).reshape(4, 2, 128).transpose(2, 1, 0))

    shared = dict(wt=wt, bias_act=bias_t, gpr=gpr_t, w1t=w1t, b1=b1, w2t=w2t,
                  b2t=b2t)
    in_maps = []
    for k in range(NCORES):
        i0 = I0S[k]
        slab = x_ext[:, :, i0:i0 + 16, :]                # [b, C, 16, 57]
        xs = np.ascontiguousarray(
            slab.reshape(B, 2, 128, 16, 57).transpose(2, 0, 1, 3, 4)
            .reshape(128, 4, 16, 57))
        m = dict(shared)
        m["xs"] = xs
        in_maps.append(m)
    return in_maps


def kernel(x, conv_w, bn_gamma, bn_beta, bn_mean, bn_var, fc1_w, fc1_b,
           fc2_w, fc2_b):
    from concourse import bass_utils

    nc = _build_program()
    in_maps = _host_prep(x, conv_w, bn_gamma, bn_beta, bn_mean, bn_var,
                         fc1_w, fc1_b, fc2_w, fc2_b)
    res = bass_utils.run_bass_kernel_spmd(nc, in_maps,
                                          core_ids=list(range(NCORES)))
    full = np.zeros((B, C, HOUT, HOUT), np.float32)
    for k in range(NCORES):
        o = res.results[k]["out"]                         # [128, 4, 18, 142]
        o = o.reshape(128, B, 2, NROWS, HOUT).transpose(1, 2, 0, 3, 4) \
             .reshape(B, C, NROWS, HOUT)
        if k < 7:
            full[:, :, 18 * k:18 * k + 18, :] = o
        else:
            full[:, :, 126:142, :] = o[:, :, 0:16, :]
    return full


# revision 17
# speedup vs baseline: 1.0115x; 1.0115x over previous
# Trainium2 Bass kernel for nn_FPN_AAR (dense_cnn): 5-branch deformable-sampled
# 3x3 conv + SKConv attention fusion, sharded over 8 NeuronCores by output rows.
#
#   - Each core computes an 18-row slab of the 142-row output for all 5 angle
#     branches (uniform geometry; core 7 overhangs and the host keeps only its
#     valid 16 rows).
#   - Offset sampling runs on-chip with constant weights: clipped sample
#     indices always land in zero padding, so a zero-margin input slab makes
#     every border case identical to the interior.
#   - Convs run as 18 accumulating fp32r matmuls per PSUM bank (9 taps x 2
#     cin chunks), N=426 (3 output rows x 142).
#   - BN+ReLU on ScalarE with per-channel bias (gamma' folded into attention
#     weights); spatial partial sums come free via accum_out.
#   - Cross-core: AllGather of per-group partial sums; every core redundantly
#     computes the tiny fc1/fc2/softmax and applies attention to its slab.
import sys
import math

sys.path.insert(0, "/opt/trn_rl_repo")

import numpy as np

KS = 3
S2 = 2**0.5
ANGLES = (0, 45, 90, 135, 180)
BN_EPS = 1e-5
NCORES = 8
B, C, H, W = 2, 256, 48, 48
HOUT = 142
NI = 7          # sample rows per core
NROWS = 18      # output rows per core slab
NG = 6          # row groups of 3
NTILE = 3 * HOUT  # 426
I0S = [6 * k for k in range(7)] + [42]


def _angle_offsets(angle):
    n = angle // 45
    if n == 0:
        ox = [0.0] * 9
        oy = [0.0] * 9
    elif n == 1:
        ox = [1 - S2, 1 - S2 * 0.5, 1, -S2 * 0.5, 0, S2 * 0.5, -1, S2 * 0.5 - 1, S2 - 1]
        oy = [1, S2 * 0.5, S2 - 1, 1 - S2 * 0.5, 0, S2 * 0.5 - 1, 1 - S2, -S2 * 0.5, -1]
    elif n == 2:
        ox = [0, 1, 2, -1, 0, 1, -2, -1, 0]
        oy = [2, 1, 0, 1, 0, -1, 0, -1, -2]
    elif n == 3:
        ox = [1, 1 + S2 * 0.5, 1 + S2, -S2 * 0.5, 0, S2 * 0.5, -1 - S2, -1 - S2 * 0.5, -1]
        oy = [1 + S2, S2 * 0.5, -1, 1 + S2 * 0.5, 0, -1 - S2 * 0.5, 1, -S2 * 0.5, 1 + S2]
    else:
        ox = [2, 2, 2, 0, 0, 0, -2, -2, -2]
        oy = [2, 0, -2, 2, 0, -2, 2, 0, -2]
    return ox, oy


def _angle_terms(angle):
    """Per kernel point p=(r,s): sampling as up to 2x2 separable terms.

    xo[c, 3I+r, 3j+s] = sum_u sum_v wx_u*wy_v * xs[c, I+dx_u+4, j+dy_v+4]
    where the input slab xs carries zero margins so clipped samples read 0,
    matching the reference's clamp-into-padding semantics exactly.
    """
    ox, oy = _angle_offsets(angle)
    pn = [-1.0, 0.0, 1.0]
    out = {}
    for p in range(9):
        cx = 1.0 + pn[p // 3] + float(ox[p])
        cy = 1.0 + pn[p % 3] + float(oy[p])
        fx, fy = math.floor(cx), math.floor(cy)
        ax, ay = cx - fx, cy - fy
        xterms = [(int(d), w) for d, w in [(fx, 1 - ax), (fx + 1, ax)] if w != 0.0]
        yterms = [(int(d), w) for d, w in [(fy, 1 - ay), (fy + 1, ay)] if w != 0.0]
        out[p] = (xterms, yterms)
    return out


_BUILD_CACHE = {}


def _build_program():
    if "nc" in _BUILD_CACHE:
        return _BUILD_CACHE["nc"]

    import concourse.bacc as bacc
    import concourse.tile as tile
    import concourse.mybir as mybir

    f32 = mybir.dt.float32
    f32r = mybir.dt.float32r
    AF = mybir.ActivationFunctionType
    ALU = mybir.AluOpType
    AX = mybir.AxisListType

    nc = bacc.Bacc("TRN2", target_bir_lowering=False, debug=False,
                   num_devices=NCORES)

    xs_d = nc.dram_tensor("xs", [128, 4, 16, 57], f32, kind="ExternalInput")
    wt_d = nc.dram_tensor("wt", [128, 2, 2, 9, 128], f32r, kind="ExternalInput")
    bias_d = nc.dram_tensor("bias_act", [128, 2, 4], f32, kind="ExternalInput")
    gpr_d = nc.dram_tensor("gpr", [128, 2, 4], f32, kind="ExternalInput")
    w1t_d = nc.dram_tensor("w1t", [128, 2, 32], f32, kind="ExternalInput")
    b1_d = nc.dram_tensor("b1", [32, 1], f32, kind="ExternalInput")
    w2t_d = nc.dram_tensor("w2t", [32, 4, 2, 128], f32, kind="ExternalInput")
    b2t_d = nc.dram_tensor("b2t", [128, 2, 4], f32, kind="ExternalInput")
    out_d = nc.dram_tensor("out", [128, 4, NROWS, HOUT], f32, kind="ExternalOutput")

    terms = {a: _angle_terms(a) for a in ANGLES}
    branch_of = {45: 0, 90: 1, 135: 2, 180: 3}
    angle_order = [45, 135, 90, 180, 0]

    with tile.TileContext(nc) as tc:
        with tc.tile_pool(name="persist", bufs=1) as pp, \
             tc.tile_pool(name="xof", bufs=1) as xofp, \
             tc.tile_pool(name="xor", bufs=3) as xorp, \
             tc.tile_pool(name="work", bufs=3) as wp, \
             tc.tile_pool(name="apply", bufs=2) as app, \
             tc.tile_pool(name="y0p", bufs=14) as y0p, \
             tc.tile_pool(name="psum", bufs=8, space="PSUM") as psp, \
             tc.tile_pool(name="dram", bufs=1, space="DRAM") as dp:

            # ---- persistent loads ----
            xs_t = []
            for _bc in range(4):
                xst = pp.tile([128, 16, 57], f32, tag=f"xs{_bc}",
                              name=f"xs{_bc}")
                nc.sync.dma_start(xst[:], xs_d.ap()[:, _bc])
                xs_t.append(xst)
            wtr_t = []
            for _oc in range(2):
                wto = pp.tile([128, 2, 9, 128], f32r, tag=f"wtr{_oc}",
                              name=f"wtr{_oc}")
                nc.sync.dma_start(wto[:], wt_d.ap()[:, :, _oc])
                wtr_t.append(wto)
            bias_sb = pp.tile([128, 2, 4], f32, tag="bias")
            nc.sync.dma_start(bias_sb[:], bias_d.ap()[:])
            gpr_sb = pp.tile([128, 2, 4], f32, tag="gpr")
            nc.sync.dma_start(gpr_sb[:], gpr_d.ap()[:])
            w1t_sb = pp.tile([128, 2, 32], f32, tag="w1t")
            nc.sync.dma_start(w1t_sb[:], w1t_d.ap()[:])
            b1_sb = pp.tile([32, 1], f32, tag="b1")
            nc.sync.dma_start(b1_sb[:], b1_d.ap()[:])
            w2t_sb = pp.tile([32, 4, 2, 128], f32, tag="w2t")
            nc.sync.dma_start(w2t_sb[:], w2t_d.ap()[:])
            b2t_sb = pp.tile([128, 2, 4], f32, tag="b2t")
            nc.sync.dma_start(b2t_sb[:], b2t_d.ap()[:])

            # PE warm-up: dependency-free matmuls on zeroed tiles run while
            # the input DMAs land, releasing the HAM clock gate before the
            # first real conv matmul.
            wz = pp.tile([128, 512], mybir.dt.bfloat16, tag="warmz")
            nc.vector.memset(wz[:], 0.0)
            wps = psp.tile([128, 512], f32, tag="ps", name="warm_ps")
            for _wi in range(40):
                nc.tensor.matmul(wps[:], wz[:, 0:128], wz[:],
                                 start=True, stop=True)

            # acc layout: [m(4), b(2), oc(2), acc(8)]
            # acc 0..4 = row-group sums g0..g4, acc 5..7 = rows 15,16,17
            acc_sb = pp.tile([128, 4, 2, 2, 8], f32, tag="acc")

            feas_dram = dp.tile([4, 2, 2, NG, 128, NTILE], f32, tag="feas_d")
            ag_in = dp.tile([128, 128], f32, tag="ag_in")
            ag_out = dp.tile([128 * NCORES, 128], f32, tag="ag_out")

            attp = pp.tile([128, 2, 2, 4], f32, tag="attp")

            # ---- helpers ----
            def emit_sample(a):
                integer_angle = all(
                    len(terms[a][p][0]) == 1 and len(terms[a][p][1]) == 1
                    for p in range(9))
                xo_r = {}
                for bb in range(2):
                    for cc in range(2):
                        xsl = xs_t[bb * 2 + cc][:]  # [128, 16, 57]
                        if integer_angle:
                            # pure shifts: write straight into the fp32r tile
                            # (raw fp32 bits are fine for the PE, like the
                            # DMA-loaded weights)
                            xr = xorp.tile([128, 21, 144], f32r,
                                           tag=f"xor{cc}",
                                           name=f"xor_{a}_{bb}_{cc}")
                            xrv = xr[:]
                            for p in range(9):
                                r, s = p // 3, p % 3
                                dx, _ = terms[a][p][0][0], None
                                dy = terms[a][p][1][0][0]
                                dx = terms[a][p][0][0][0]
                                nc.gpsimd.tensor_copy(
                                    xrv[:, r::3, s::3],
                                    xsl[:, 4 + dx:4 + dx + 7,
                                        4 + dy:4 + dy + 48])
                            xo_r[(bb, cc)] = xr
                            continue
                        xof = xofp.tile([128, 21, 144], f32, tag="xof",
                                        name=f"xof_{a}_{bb}_{cc}")
                        for p in range(9):
                            r, s = p // 3, p % 3
                            xterms, yterms = terms[a][p]
                            dst = xof[:, r::3, s::3]    # [128, 7, 48]
                            if len(xterms) == 1 and len(yterms) == 1:
                                dx, wx = xterms[0]
                                dy, wy = yterms[0]
                                src = xsl[:, 4 + dx:4 + dx + 7, 4 + dy:4 + dy + 48]
                                w = wx * wy
                                if w == 1.0:
                                    nc.gpsimd.tensor_copy(dst, src)
                                else:
                                    nc.gpsimd.tensor_scalar_mul(dst, src, float(w))
                            else:
                                if len(xterms) == 1:
                                    dx, wx = xterms[0]
                                    At = xsl[:, 4 + dx:4 + dx + 7, :]
                                    ascale = wx
                                else:
                                    (dx0, wx0), (dx1, wx1) = xterms
                                    Atile = wp.tile([128, 7, 57], f32, tag="stepA", bufs=2,
                                                    name=f"sa_{a}_{bb}_{cc}_{p}")
                                    t2 = wp.tile([128, 7, 57], f32, tag="stepA2", bufs=2,
                                                 name=f"sa2_{a}_{bb}_{cc}_{p}")
                                    nc.vector.tensor_scalar_mul(
                                        Atile[:], xsl[:, 4 + dx0:4 + dx0 + 7, :],
                                        float(wx0))
                                    nc.vector.tensor_scalar_mul(
                                        t2[:], xsl[:, 4 + dx1:4 + dx1 + 7, :],
                                        float(wx1))
                                    nc.vector.tensor_add(Atile[:], Atile[:], t2[:])
                                    At = Atile[:]
                                    ascale = 1.0
                                if len(yterms) == 1:
                                    dy, wy = yterms[0]
                                    w = ascale * wy
                                    src = At[:, :, 4 + dy:4 + dy + 48]
                                    if w == 1.0:
                                        nc.scalar.copy(dst, src)
                                    else:
                                        nc.scalar.mul(dst, src, float(w))
                                else:
                                    (dy0, wy0), (dy1, wy1) = yterms
                                    tb = wp.tile([128, 7, 48], f32, tag="stepB", bufs=2,
                                                 name=f"sb_{a}_{bb}_{cc}_{p}")
                                    nc.vector.tensor_scalar_mul(
                                        dst, At[:, :, 4 + dy0:4 + dy0 + 48],
                                        float(ascale * wy0))
                                    nc.vector.tensor_scalar_mul(
                                        tb[:], At[:, :, 4 + dy1:4 + dy1 + 48],
                                        float(ascale * wy1))
                                    nc.vector.tensor_add(dst, dst, tb[:])
                        xr = xorp.tile([128, 21, 144], f32r, tag=f"xor{cc}",
                                       name=f"xor_{a}_{bb}_{cc}")
                        nc.vector.tensor_copy(xr[:], xof[:])
                        xo_r[(bb, cc)] = xr
                return xo_r

            def emit_conv(a, bb, oc, xo_r):
                banks = [psp.tile([128, 3, HOUT], f32, tag="ps",
                                  name=f"ps_{a}_{bb}_{oc}_{g}")
                         for g in range(NG)]
                for tap in range(9):
                    di, dj = tap // 3, tap % 3
                    for cc in range(2):
                        lhsT = wtr_t[oc][:, cc, tap, :]
                        xr = xo_r[(bb, cc)]
                        first = (tap == 0 and cc == 0)
                        last = (tap == 8 and cc == 1)
                        for g in range(NG):
                            rhs = xr[:, 3 * g + di:3 * g + di + 3, dj:dj + HOUT]
                            nc.tensor.matmul(banks[g][:], lhsT, rhs,
                                             start=first, stop=last)
                return banks

            def emit_drain_branch(a, bb, oc, banks):
                m = branch_of[a]
                for g in range(NG):
                    ft = wp.tile([128, 3, HOUT], f32, tag="feas_t",
                                 name=f"ft_{a}_{bb}_{oc}_{g}")
                    if g < 5:
                        nc.scalar.activation(
                            ft[:], banks[g][:], AF.Relu,
                            bias=bias_sb[:, oc, m:m + 1],
                            scale=gpr_sb[:, oc, m:m + 1],
                            accum_out=acc_sb[:, m, bb, oc, g:g + 1])
                    else:
                        for r in range(3):
                            nc.scalar.activation(
                                ft[:, r, :], banks[g][:, r, :], AF.Relu,
                                bias=bias_sb[:, oc, m:m + 1],
                                scale=gpr_sb[:, oc, m:m + 1],
                                accum_out=acc_sb[:, m, bb, oc, 5 + r:6 + r])
                    nc.sync.dma_start(feas_dram[m, bb, oc, g, :, :],
                                      ft[:].rearrange("p a b -> p (a b)"))

            def emit_drain_y0(bb, oc, banks):
                tiles = []
                for g in range(NG):
                    yt = y0p.tile([128, 3, HOUT], f32, tag="y0sb",
                                  name=f"y0_{bb}_{oc}_{g}")
                    nc.scalar.copy(yt[:], banks[g][:])
                    tiles.append(yt)
                return tiles

            def emit_collective_reduce():
                nc.sync.dma_start(
                    ag_in[:, :],
                    acc_sb[:].rearrange("p a b c d -> p (a b c d)"))
                nc.gpsimd.collective_compute(
                    "AllGather", ALU.bypass,
                    replica_groups=[list(range(NCORES))],
                    ins=[ag_in.opt()], outs=[ag_out.opt()])
                ag_sb = pp.tile([128, NCORES, 16, 8], f32, tag="ag_sb")
                nc.sync.dma_start(
                    ag_sb[:],
                    ag_out[:].rearrange("(s p) (mbo a) -> p s mbo a",
                                        p=128, a=8))
                r1 = pp.tile([128, NCORES, 16], f32, tag="r1")
                nc.vector.reduce_sum(r1[:], ag_sb[:, :, :, 0:6], axis=AX.X)
                r2 = pp.tile([128, NCORES - 1, 16], f32, tag="r2")
                nc.vector.reduce_sum(r2[:], ag_sb[:, 0:7, :, 6:8], axis=AX.X)
                fm = pp.tile([128, 16], f32, tag="fm")
                fm2 = pp.tile([128, 16], f32, tag="fm2")
                nc.vector.reduce_sum(fm[:], r1[:].transpose([0, 2, 1]),
                                     axis=AX.X)
                nc.vector.reduce_sum(fm2[:], r2[:].transpose([0, 2, 1]),
                                     axis=AX.X)
                nc.vector.tensor_add(fm[:], fm[:], fm2[:])
                # fm layout [m, b, oc]; feas sums are already BN'd
                fmv = fm[:].rearrange("p (m b o) -> p m b o", m=4, b=2)
                fs = pp.tile([128, 2, 2], f32, tag="fs")
                nc.vector.reduce_sum(fs[:], fmv.transpose([0, 2, 3, 1]),
                                     axis=AX.X)
                nc.vector.tensor_scalar_mul(fs[:], fs[:], 1.0 / (HOUT * HOUT))
                return fs

            def emit_fc_softmax(fs):
                pz = psp.tile([32, 2], f32, tag="ps", name="pz_fc1")
                for cc in range(2):
                    nc.tensor.matmul(pz[:], w1t_sb[:, cc, :], fs[:, :, cc],
                                     start=(cc == 0), stop=(cc == 1))
                zt = pp.tile([32, 2], f32, tag="zt")
                nc.scalar.add(zt[:], pz[:], b1_sb[:, 0:1])
                logit = pp.tile([128, 2, 2, 4], f32, tag="logit")
                for m in range(4):
                    for oc in range(2):
                        p2 = psp.tile([128, 2], f32, tag="ps",
                                      name=f"p2_fc2_{m}_{oc}")
                        nc.tensor.matmul(p2[:], w2t_sb[:, m, oc, :], zt[:],
                                         start=True, stop=True)
                        nc.scalar.add(logit[:, oc, :, m], p2[:],
                                      b2t_sb[:, oc, m:m + 1])
                rmax = pp.tile([128, 2, 2, 1], f32, tag="rmax")
                nc.vector.tensor_reduce(rmax[:], logit[:], AX.X, ALU.max)
                nc.vector.tensor_tensor(
                    logit[:], logit[:],
                    rmax[:].broadcast_to([128, 2, 2, 4]), ALU.subtract)
                elog = pp.tile([128, 2, 2, 4], f32, tag="elog")
                nc.scalar.activation(elog[:], logit[:], AF.Exp)
                ssum = pp.tile([128, 2, 2, 1], f32, tag="ssum")
                nc.vector.reduce_sum(ssum[:], elog[:], axis=AX.X)
                rinv = pp.tile([128, 2, 2, 1], f32, tag="rinv")
                nc.vector.reciprocal(rinv[:], ssum[:])
                nc.vector.tensor_tensor(
                    attp[:], elog[:],
                    rinv[:].broadcast_to([128, 2, 2, 4]), ALU.mult)

            def emit_apply(bb, oc, y0sb):
                for g in range(NG):
                    fts = []
                    for m in range(4):
                        ft = app.tile([128, NTILE], f32, tag=f"ld{m}",
                                      bufs=3, name=f"ld_{bb}_{oc}_{g}_{m}")
                        nc.sync.dma_start(ft[:], feas_dram[m, bb, oc, g, :, :])
                        fts.append(ft)
                    acc = app.tile([128, NTILE], f32, tag="acc_t",
                                   name=f"acc_{bb}_{oc}_{g}")
                    tmp = app.tile([128, NTILE], f32, tag="tmp_t", bufs=1,
                                   name=f"tmp_{bb}_{oc}_{g}")
                    t1 = app.tile([128, NTILE], f32, tag="t1_t", bufs=1,
                                  name=f"t1_{bb}_{oc}_{g}")
                    t2 = app.tile([128, NTILE], f32, tag="t2_t", bufs=1,
                                  name=f"t2_{bb}_{oc}_{g}")
                    # scale the four branches on three different engines
                    nc.vector.tensor_scalar_mul(
                        acc[:], fts[0][:], attp[:, oc, bb, 0:1])
                    nc.scalar.mul(t1[:], fts[1][:], attp[:, oc, bb, 1:2])
                    nc.gpsimd.tensor_scalar_mul(
                        t2[:], fts[2][:], attp[:, oc, bb, 2:3])
                    nc.vector.tensor_scalar_mul(
                        tmp[:], fts[3][:], attp[:, oc, bb, 3:4])
                    nc.vector.tensor_add(acc[:], acc[:], t1[:])
                    nc.gpsimd.tensor_add(
                        t2[:], t2[:],
                        y0sb[g][:].rearrange("p a b -> p (a b)"))
                    nc.vector.tensor_add(tmp[:], tmp[:], t2[:])
                    nc.vector.tensor_add(acc[:], acc[:], tmp[:])
                    nc.sync.dma_start(
                        out_d.ap()[:, bb * 2 + oc, 3 * g:3 * g + 3, :],
                        acc[:].rearrange("p (a b) -> p a b", a=3))

            # ---- main schedule ----
            # BN branches first; collective overlaps angle-0 convs; fc matmuls
            # sit after two angle-0 phases so the in-order PE queue never
            # stalls on the collective; applies interleave with the remaining
            # angle-0 phases.
            for a in [90, 45, 135, 180]:
                xo_r = emit_sample(a)
                for bb in range(2):
                    for oc in range(2):
                        banks = emit_conv(a, bb, oc, xo_r)
                        emit_drain_branch(a, bb, oc, banks)

            fs = emit_collective_reduce()

            xo0 = emit_sample(0)
            y0sb = {}
            y0sb[(0, 0)] = emit_drain_y0(0, 0, emit_conv(0, 0, 0, xo0))
            emit_fc_softmax(fs)
            y0sb[(0, 1)] = emit_drain_y0(0, 1, emit_conv(0, 0, 1, xo0))
            emit_apply(0, 0, y0sb[(0, 0)])
            y0sb[(1, 0)] = emit_drain_y0(1, 0, emit_conv(0, 1, 0, xo0))
            emit_apply(0, 1, y0sb[(0, 1)])
            y0sb[(1, 1)] = emit_drain_y0(1, 1, emit_conv(0, 1, 1, xo0))
            emit_apply(1, 0, y0sb[(1, 0)])
            emit_apply(1, 1, y0sb[(1, 1)])

    nc.compile()
    _BUILD_CACHE["nc"] = nc
    return nc


def _host_prep(x, conv_w, bn_gamma, bn_beta, bn_mean, bn_var, fc1_w, fc1_b,
               fc2_w, fc2_b):
    x = np.asarray(x, np.float32)
    conv_w = np.asarray(conv_w, np.float32)
    x_ext = np.zeros((B, C, 60, 57), np.float32)
    x_ext[:, :, 5:53, 5:53] = x

    gprime = (np.asarray(bn_gamma) / np.sqrt(np.asarray(bn_var) + BN_EPS)) \
        .astype(np.float32)
    bprime = (np.asarray(bn_beta) - np.asarray(bn_mean) * gprime) \
        .astype(np.float32)
    bias_act = bprime                                    # [4, 256]

    # wt[ci, cc, oc, tap, co] = conv_w[oc*128+co, cc*128+ci, di, dj]
    w6 = conv_w.reshape(2, 128, 2, 128, 9)               # [oc, co, cc, ci, tap]
    wt = np.ascontiguousarray(w6.transpose(3, 2, 0, 4, 1))

    bias_t = np.ascontiguousarray(
        bias_act.reshape(4, 2, 128).transpose(2, 1, 0))  # [co, oc, m]
    gpr_t = np.ascontiguousarray(
        gprime.reshape(4, 2, 128).transpose(2, 1, 0))
    w1t = np.ascontiguousarray(
        np.asarray(fc1_w, np.float32).T.reshape(2, 128, 32).transpose(1, 0, 2))
    b1 = np.asarray(fc1_b, np.float32).reshape(32, 1).copy()
    w2t = np.ascontiguousarray(
        np.asarray(fc2_w, np.float32).reshape(4, 2, 128, 32)
        .transpose(3, 0, 1, 2))
    b2t = np.ascontiguousarray(
        np.asarray(fc2_b, np.float32).reshape(4, 2, 128).transpose(2, 1, 0))

    shared = dict(wt=wt, bias_act=bias_t, gpr=gpr_t, w1t=w1t, b1=b1, w2t=w2t,
                  b2t=b2t)
    in_maps = []
    for k in range(NCORES):
        i0 = I0S[k]
        slab = x_ext[:, :, i0:i0 + 16, :]                # [b, C, 16, 57]
        xs = np.ascontiguousarray(
            slab.reshape(B, 2, 128, 16, 57).transpose(2, 0, 1, 3, 4)
            .reshape(128, 4, 16, 57))
        m = dict(shared)
        m["xs"] = xs
        in_maps.append(m)
    return in_maps


def kernel(x, conv_w, bn_gamma, bn_beta, bn_mean, bn_var, fc1_w, fc1_b,
           fc2_w, fc2_b):
    from concourse import bass_utils

    nc = _build_program()
    in_maps = _host_prep(x, conv_w, bn_gamma, bn_beta, bn_mean, bn_var,
                         fc1_w, fc1_b, fc2_w, fc2_b)
    res = bass_utils.run_bass_kernel_spmd(nc, in_maps,
                                          core_ids=list(range(NCORES)))
    full = np.zeros((B, C, HOUT, HOUT), np.float32)
    for k in range(NCORES):
        o = res.results[k]["out"]                         # [128, 4, 18, 142]
        o = o.reshape(128, B, 2, NROWS, HOUT).transpose(1, 2, 0, 3, 4) \
             .reshape(B, C, NROWS, HOUT)
        if k < 7:
            full[:, :, 18 * k:18 * k + 18, :] = o
        else:
            full[:, :, 126:142, :] = o[:, :, 0:16, :]
    return full


# revision 21
# speedup vs baseline: 1.0168x; 1.0052x over previous
# Trainium2 Bass kernel for nn_FPN_AAR (dense_cnn): 5-branch deformable-sampled
# 3x3 conv + SKConv attention fusion, sharded over 8 NeuronCores by output rows.
#
#   - Each core computes an 18-row slab of the 142-row output for all 5 angle
#     branches (uniform geometry; core 7 overhangs and the host keeps only its
#     valid 16 rows).
#   - Offset sampling runs on-chip with constant weights: clipped sample
#     indices always land in zero padding, so a zero-margin input slab makes
#     every border case identical to the interior.
#   - Convs run as 18 accumulating fp32r matmuls per PSUM bank (9 taps x 2
#     cin chunks), N=426 (3 output rows x 142).
#   - BN+ReLU on ScalarE with per-channel bias (gamma' folded into attention
#     weights); spatial partial sums come free via accum_out.
#   - Cross-core: AllGather of per-group partial sums; every core redundantly
#     computes the tiny fc1/fc2/softmax and applies attention to its slab.
import sys
import math

sys.path.insert(0, "/opt/trn_rl_repo")

import numpy as np

KS = 3
S2 = 2**0.5
ANGLES = (0, 45, 90, 135, 180)
BN_EPS = 1e-5
NCORES = 8
B, C, H, W = 2, 256, 48, 48
HOUT = 142
NI = 7          # sample rows per core
NROWS = 18      # output rows per core slab
NG = 6          # row groups of 3
NTILE = 3 * HOUT  # 426
I0S = [6 * k for k in range(7)] + [42]


def _angle_offsets(angle):
    n = angle // 45
    if n == 0:
        ox = [0.0] * 9
        oy = [0.0] * 9
    elif n == 1:
        ox = [1 - S2, 1 - S2 * 0.5, 1, -S2 * 0.5, 0, S2 * 0.5, -1, S2 * 0.5 - 1, S2 - 1]
        oy = [1, S2 * 0.5, S2 - 1, 1 - S2 * 0.5, 0, S2 * 0.5 - 1, 1 - S2, -S2 * 0.5, -1]
    elif n == 2:
        ox = [0, 1, 2, -1, 0, 1, -2, -1, 0]
        oy = [2, 1, 0, 1, 0, -1, 0, -1, -2]
    elif n == 3:
        ox = [1, 1 + S2 * 0.5, 1 + S2, -S2 * 0.5, 0, S2 * 0.5, -1 - S2, -1 - S2 * 0.5, -1]
        oy = [1 + S2, S2 * 0.5, -1, 1 + S2 * 0.5, 0, -1 - S2 * 0.5, 1, -S2 * 0.5, 1 + S2]
    else:
        ox = [2, 2, 2, 0, 0, 0, -2, -2, -2]
        oy = [2, 0, -2, 2, 0, -2, 2, 0, -2]
    return ox, oy


def _angle_terms(angle):
    """Per kernel point p=(r,s): sampling as up to 2x2 separable terms.

    xo[c, 3I+r, 3j+s] = sum_u sum_v wx_u*wy_v * xs[c, I+dx_u+4, j+dy_v+4]
    where the input slab xs carries zero margins so clipped samples read 0,
    matching the reference's clamp-into-padding semantics exactly.
    """
    ox, oy = _angle_offsets(angle)
    pn = [-1.0, 0.0, 1.0]
    out = {}
    for p in range(9):
        cx = 1.0 + pn[p // 3] + float(ox[p])
        cy = 1.0 + pn[p % 3] + float(oy[p])
        fx, fy = math.floor(cx), math.floor(cy)
        ax, ay = cx - fx, cy - fy
        xterms = [(int(d), w) for d, w in [(fx, 1 - ax), (fx + 1, ax)] if w != 0.0]
        yterms = [(int(d), w) for d, w in [(fy, 1 - ay), (fy + 1, ay)] if w != 0.0]
        out[p] = (xterms, yterms)
    return out


_BUILD_CACHE = {}


def _build_program():
    if "nc" in _BUILD_CACHE:
        return _BUILD_CACHE["nc"]

    import concourse.bacc as bacc
    import concourse.tile as tile
    import concourse.mybir as mybir

    f32 = mybir.dt.float32
    f32r = mybir.dt.float32r
    AF = mybir.ActivationFunctionType
    ALU = mybir.AluOpType
    AX = mybir.AxisListType

    nc = bacc.Bacc("TRN2", target_bir_lowering=False, debug=False,
                   num_devices=NCORES)

    xs_d = nc.dram_tensor("xs", [128, 4, 16, 57], f32, kind="ExternalInput")
    wt_d = nc.dram_tensor("wt", [128, 2, 2, 9, 128], f32r, kind="ExternalInput")
    bias_d = nc.dram_tensor("bias_act", [128, 2, 4], f32, kind="ExternalInput")
    gpr_d = nc.dram_tensor("gpr", [128, 2, 4], f32, kind="ExternalInput")
    w1t_d = nc.dram_tensor("w1t", [128, 2, 32], f32, kind="ExternalInput")
    b1_d = nc.dram_tensor("b1", [32, 1], f32, kind="ExternalInput")
    w2t_d = nc.dram_tensor("w2t", [32, 4, 2, 128], f32, kind="ExternalInput")
    b2t_d = nc.dram_tensor("b2t", [128, 2, 4], f32, kind="ExternalInput")
    out_d = nc.dram_tensor("out", [128, 4, NROWS, HOUT], f32, kind="ExternalOutput")

    terms = {a: _angle_terms(a) for a in ANGLES}
    branch_of = {45: 0, 90: 1, 135: 2, 180: 3}
    angle_order = [45, 135, 90, 180, 0]

    with tile.TileContext(nc) as tc:
        with tc.tile_pool(name="persist", bufs=1) as pp, \
             tc.tile_pool(name="xof", bufs=1) as xofp, \
             tc.tile_pool(name="xor", bufs=3) as xorp, \
             tc.tile_pool(name="work", bufs=3) as wp, \
             tc.tile_pool(name="apply", bufs=2) as app, \
             tc.tile_pool(name="y0p", bufs=12) as y0p, \
             tc.tile_pool(name="psum", bufs=8, space="PSUM") as psp, \
             tc.tile_pool(name="dram", bufs=1, space="DRAM") as dp:

            # ---- persistent loads ----
            xs_t = []
            for _bc in range(4):
                xst = pp.tile([128, 16, 57], f32, tag=f"xs{_bc}",
                              name=f"xs{_bc}")
                nc.sync.dma_start(xst[:], xs_d.ap()[:, _bc])
                xs_t.append(xst)
            wtr_t = []
            for _oc in range(2):
                wto = pp.tile([128, 2, 9, 128], f32r, tag=f"wtr{_oc}",
                              name=f"wtr{_oc}")
                nc.sync.dma_start(wto[:], wt_d.ap()[:, :, _oc])
                wtr_t.append(wto)
            bias_sb = pp.tile([128, 2, 4], f32, tag="bias")
            nc.sync.dma_start(bias_sb[:], bias_d.ap()[:])
            gpr_sb = pp.tile([128, 2, 4], f32, tag="gpr")
            nc.sync.dma_start(gpr_sb[:], gpr_d.ap()[:])
            w1t_sb = pp.tile([128, 2, 32], f32, tag="w1t")
            nc.sync.dma_start(w1t_sb[:], w1t_d.ap()[:])
            b1_sb = pp.tile([32, 1], f32, tag="b1")
            nc.sync.dma_start(b1_sb[:], b1_d.ap()[:])
            w2t_sb = pp.tile([32, 4, 2, 128], f32, tag="w2t")
            nc.sync.dma_start(w2t_sb[:], w2t_d.ap()[:])
            b2t_sb = pp.tile([128, 2, 4], f32, tag="b2t")
            nc.sync.dma_start(b2t_sb[:], b2t_d.ap()[:])

            # PE warm-up: dependency-free matmuls on zeroed tiles run while
            # the input DMAs land, releasing the HAM clock gate before the
            # first real conv matmul.
            wz = pp.tile([128, 512], mybir.dt.bfloat16, tag="warmz")
            nc.vector.memset(wz[:], 0.0)
            wps = psp.tile([128, 512], f32, tag="ps", name="warm_ps")
            for _wi in range(40):
                nc.tensor.matmul(wps[:], wz[:, 0:128], wz[:],
                                 start=True, stop=True)

            # acc layout: [m(4), b(2), oc(2), acc(8)]
            # acc 0..4 = row-group sums g0..g4, acc 5..7 = rows 15,16,17
            acc_sb = pp.tile([128, 4, 2, 2, 8], f32, tag="acc")

            feas_dram = dp.tile([4, 2, 2, NG, 128, NTILE], f32, tag="feas_d")
            ag_in = dp.tile([128, 128], f32, tag="ag_in")
            ag_out = dp.tile([128 * NCORES, 128], f32, tag="ag_out")

            attp = pp.tile([128, 2, 2, 4], f32, tag="attp")

            # ---- helpers ----
            def emit_sample(a):
                integer_angle = all(
                    len(terms[a][p][0]) == 1 and len(terms[a][p][1]) == 1
                    for p in range(9))
                xo_r = {}
                for bb in range(2):
                    for cc in range(2):
                        xsl = xs_t[bb * 2 + cc][:]  # [128, 16, 57]
                        if integer_angle:
                            # pure shifts: write straight into the fp32r tile
                            # (raw fp32 bits are fine for the PE, like the
                            # DMA-loaded weights)
                            xr = xorp.tile([128, 21, 144], f32r,
                                           tag=f"xor{cc}",
                                           name=f"xor_{a}_{bb}_{cc}")
                            xrv = xr[:]
                            for p in range(9):
                                r, s = p // 3, p % 3
                                dx, _ = terms[a][p][0][0], None
                                dy = terms[a][p][1][0][0]
                                dx = terms[a][p][0][0][0]
                                nc.gpsimd.tensor_copy(
                                    xrv[:, r::3, s::3],
                                    xsl[:, 4 + dx:4 + dx + 7,
                                        4 + dy:4 + dy + 48])
                            xo_r[(bb, cc)] = xr
                            continue
                        xof = xofp.tile([128, 21, 144], f32, tag="xof",
                                        name=f"xof_{a}_{bb}_{cc}")
                        for p in range(9):
                            r, s = p // 3, p % 3
                            xterms, yterms = terms[a][p]
                            dst = xof[:, r::3, s::3]    # [128, 7, 48]
                            if len(xterms) == 1 and len(yterms) == 1:
                                dx, wx = xterms[0]
                                dy, wy = yterms[0]
                                src = xsl[:, 4 + dx:4 + dx + 7, 4 + dy:4 + dy + 48]
                                w = wx * wy
                                if w == 1.0:
                                    nc.gpsimd.tensor_copy(dst, src)
                                else:
                                    nc.gpsimd.tensor_scalar_mul(dst, src, float(w))
                            else:
                                if len(xterms) == 1:
                                    dx, wx = xterms[0]
                                    At = xsl[:, 4 + dx:4 + dx + 7, :]
                                    ascale = wx
                                else:
                                    (dx0, wx0), (dx1, wx1) = xterms
                                    Atile = wp.tile([128, 7, 57], f32, tag="stepA", bufs=1,
                                                    name=f"sa_{a}_{bb}_{cc}_{p}")
                                    t2 = wp.tile([128, 7, 57], f32, tag="stepA2", bufs=1,
                                                 name=f"sa2_{a}_{bb}_{cc}_{p}")
                                    nc.vector.tensor_scalar_mul(
                                        Atile[:], xsl[:, 4 + dx0:4 + dx0 + 7, :],
                                        float(wx0))
                                    nc.vector.tensor_scalar_mul(
                                        t2[:], xsl[:, 4 + dx1:4 + dx1 + 7, :],
                                        float(wx1))
                                    nc.vector.tensor_add(Atile[:], Atile[:], t2[:])
                                    At = Atile[:]
                                    ascale = 1.0
                                if len(yterms) == 1:
                                    dy, wy = yterms[0]
                                    w = ascale * wy
                                    src = At[:, :, 4 + dy:4 + dy + 48]
                                    if w == 1.0:
                                        nc.scalar.copy(dst, src)
                                    else:
                                        nc.scalar.mul(dst, src, float(w))
                                else:
                                    (dy0, wy0), (dy1, wy1) = yterms
                                    tb = wp.tile([128, 7, 48], f32, tag="stepB", bufs=1,
                                                 name=f"sb_{a}_{bb}_{cc}_{p}")
                                    nc.vector.tensor_scalar_mul(
                                        dst, At[:, :, 4 + dy0:4 + dy0 + 48],
                                        float(ascale * wy0))
                                    nc.vector.tensor_scalar_mul(
                                        tb[:], At[:, :, 4 + dy1:4 + dy1 + 48],
                                        float(ascale * wy1))
                                    nc.vector.tensor_add(dst, dst, tb[:])
                        xr = xorp.tile([128, 21, 144], f32r, tag=f"xor{cc}",
                                       name=f"xor_{a}_{bb}_{cc}")
                        nc.vector.tensor_copy(xr[:], xof[:])
                        xo_r[(bb, cc)] = xr
                return xo_r

            def emit_conv(a, bb, oc, xo_r):
                banks = [psp.tile([128, 3, HOUT], f32, tag="ps",
                                  name=f"ps_{a}_{bb}_{oc}_{g}")
                         for g in range(NG)]
                for tap in range(9):
                    di, dj = tap // 3, tap % 3
                    for cc in range(2):
                        lhsT = wtr_t[oc][:, cc, tap, :]
                        xr = xo_r[(bb, cc)]
                        first = (tap == 0 and cc == 0)
                        last = (tap == 8 and cc == 1)
                        for g in range(NG):
                            rhs = xr[:, 3 * g + di:3 * g + di + 3, dj:dj + HOUT]
                            nc.tensor.matmul(banks[g][:], lhsT, rhs,
                                             start=first, stop=last)
                return banks

            def emit_drain_branch(a, bb, oc, banks):
                m = branch_of[a]
                for g in range(NG):
                    ft = wp.tile([128, 3, HOUT], f32, tag="feas_t",
                                 name=f"ft_{a}_{bb}_{oc}_{g}")
                    if g < 5:
                        nc.scalar.activation(
                            ft[:], banks[g][:], AF.Relu,
                            bias=bias_sb[:, oc, m:m + 1],
                            scale=gpr_sb[:, oc, m:m + 1],
                            accum_out=acc_sb[:, m, bb, oc, g:g + 1])
                    else:
                        for r in range(3):
                            nc.scalar.activation(
                                ft[:, r, :], banks[g][:, r, :], AF.Relu,
                                bias=bias_sb[:, oc, m:m + 1],
                                scale=gpr_sb[:, oc, m:m + 1],
                                accum_out=acc_sb[:, m, bb, oc, 5 + r:6 + r])
                    nc.sync.dma_start(feas_dram[m, bb, oc, g, :, :],
                                      ft[:].rearrange("p a b -> p (a b)"))

            def emit_drain_y0(bb, oc, banks):
                tiles = []
                for g in range(NG):
                    yt = y0p.tile([128, 3, HOUT], f32, tag="y0sb",
                                  name=f"y0_{bb}_{oc}_{g}")
                    nc.scalar.copy(yt[:], banks[g][:])
                    tiles.append(yt)
                return tiles

            def emit_collective_reduce():
                nc.sync.dma_start(
                    ag_in[:, :],
                    acc_sb[:].rearrange("p a b c d -> p (a b c d)"))
                nc.gpsimd.collective_compute(
                    "AllGather", ALU.bypass,
                    replica_groups=[list(range(NCORES))],
                    ins=[ag_in.opt()], outs=[ag_out.opt()])
                ag_sb = pp.tile([128, NCORES, 16, 8], f32, tag="ag_sb")
                nc.sync.dma_start(
                    ag_sb[:],
                    ag_out[:].rearrange("(s p) (mbo a) -> p s mbo a",
                                        p=128, a=8))
                r1 = pp.tile([128, NCORES, 16], f32, tag="r1")
                nc.vector.reduce_sum(r1[:], ag_sb[:, :, :, 0:6], axis=AX.X)
                r2 = pp.tile([128, NCORES - 1, 16], f32, tag="r2")
                nc.vector.reduce_sum(r2[:], ag_sb[:, 0:7, :, 6:8], axis=AX.X)
                fm = pp.tile([128, 16], f32, tag="fm")
                fm2 = pp.tile([128, 16], f32, tag="fm2")
                nc.vector.reduce_sum(fm[:], r1[:].transpose([0, 2, 1]),
                                     axis=AX.X)
                nc.vector.reduce_sum(fm2[:], r2[:].transpose([0, 2, 1]),
                                     axis=AX.X)
                nc.vector.tensor_add(fm[:], fm[:], fm2[:])
                # fm layout [m, b, oc]; feas sums are already BN'd
                fmv = fm[:].rearrange("p (m b o) -> p m b o", m=4, b=2)
                fs = pp.tile([128, 2, 2], f32, tag="fs")
                nc.vector.reduce_sum(fs[:], fmv.transpose([0, 2, 3, 1]),
                                     axis=AX.X)
                nc.vector.tensor_scalar_mul(fs[:], fs[:], 1.0 / (HOUT * HOUT))
                return fs

            def emit_fc_softmax(fs):
                pz = psp.tile([32, 2], f32, tag="ps", name="pz_fc1")
                for cc in range(2):
                    nc.tensor.matmul(pz[:], w1t_sb[:, cc, :], fs[:, :, cc],
                                     start=(cc == 0), stop=(cc == 1))
                zt = pp.tile([32, 2], f32, tag="zt")
                nc.scalar.add(zt[:], pz[:], b1_sb[:, 0:1])
                logit = pp.tile([128, 2, 2, 4], f32, tag="logit")
                for m in range(4):
                    for oc in range(2):
                        p2 = psp.tile([128, 2], f32, tag="ps",
                                      name=f"p2_fc2_{m}_{oc}")
                        nc.tensor.matmul(p2[:], w2t_sb[:, m, oc, :], zt[:],
                                         start=True, stop=True)
                        nc.scalar.add(logit[:, oc, :, m], p2[:],
                                      b2t_sb[:, oc, m:m + 1])
                rmax = pp.tile([128, 2, 2, 1], f32, tag="rmax")
                nc.vector.tensor_reduce(rmax[:], logit[:], AX.X, ALU.max)
                nc.vector.tensor_tensor(
                    logit[:], logit[:],
                    rmax[:].broadcast_to([128, 2, 2, 4]), ALU.subtract)
                elog = pp.tile([128, 2, 2, 4], f32, tag="elog")
                nc.scalar.activation(elog[:], logit[:], AF.Exp)
                ssum = pp.tile([128, 2, 2, 1], f32, tag="ssum")
                nc.vector.reduce_sum(ssum[:], elog[:], axis=AX.X)
                rinv = pp.tile([128, 2, 2, 1], f32, tag="rinv")
                nc.vector.reciprocal(rinv[:], ssum[:])
                nc.vector.tensor_tensor(
                    attp[:], elog[:],
                    rinv[:].broadcast_to([128, 2, 2, 4]), ALU.mult)

            def emit_apply(bb, oc, y0sb):
                # pass 1: att-weighted branch sums (independent of angle-0)
                accs = []
                for g in range(NG):
                    fts = []
                    for m in range(4):
                        ft = app.tile([128, NTILE], f32, tag=f"ld{m}",
                                      bufs=3, name=f"ld_{bb}_{oc}_{g}_{m}")
                        nc.sync.dma_start(ft[:], feas_dram[m, bb, oc, g, :, :])
                        fts.append(ft)
                    acc = app.tile([128, NTILE], f32, tag="acc_t", bufs=6,
                                   name=f"acc_{bb}_{oc}_{g}")
                    tmp = app.tile([128, NTILE], f32, tag="tmp_t", bufs=1,
                                   name=f"tmp_{bb}_{oc}_{g}")
                    t1 = app.tile([128, NTILE], f32, tag="t1_t", bufs=1,
                                  name=f"t1_{bb}_{oc}_{g}")
                    t2 = app.tile([128, NTILE], f32, tag="t2_t", bufs=1,
                                  name=f"t2_{bb}_{oc}_{g}")
                    nc.vector.tensor_scalar_mul(
                        acc[:], fts[0][:], attp[:, oc, bb, 0:1])
                    nc.scalar.mul(t1[:], fts[1][:], attp[:, oc, bb, 1:2])
                    nc.gpsimd.tensor_scalar_mul(
                        t2[:], fts[2][:], attp[:, oc, bb, 2:3])
                    nc.vector.tensor_scalar_mul(
                        tmp[:], fts[3][:], attp[:, oc, bb, 3:4])
                    nc.vector.tensor_add(acc[:], acc[:], t1[:])
                    nc.vector.tensor_add(tmp[:], tmp[:], t2[:])
                    nc.vector.tensor_add(acc[:], acc[:], tmp[:])
                    accs.append(acc)
                # pass 2: only these trailing adds wait on the angle-0 drains
                for g in range(NG):
                    nc.gpsimd.tensor_add(
                        accs[g][:], accs[g][:],
                        y0sb[g][:].rearrange("p a b -> p (a b)"))
                    nc.sync.dma_start(
                        out_d.ap()[:, bb * 2 + oc, 3 * g:3 * g + 3, :],
                        accs[g][:].rearrange("p (a b) -> p a b", a=3))

            # ---- main schedule ----
            # BN branches first; collective overlaps angle-0 convs; fc matmuls
            # sit after two angle-0 phases so the in-order PE queue never
            # stalls on the collective; applies interleave with the remaining
            # angle-0 phases.
            for a in [90, 45, 135, 180]:
                xo_r = emit_sample(a)
                for bb in range(2):
                    for oc in range(2):
                        banks = emit_conv(a, bb, oc, xo_r)
                        emit_drain_branch(a, bb, oc, banks)

            fs = emit_collective_reduce()

            xo0 = emit_sample(0)
            y0sb = {}
            y0sb[(0, 0)] = emit_drain_y0(0, 0, emit_conv(0, 0, 0, xo0))
            emit_fc_softmax(fs)
            y0sb[(0, 1)] = emit_drain_y0(0, 1, emit_conv(0, 0, 1, xo0))
            emit_apply(0, 0, y0sb[(0, 0)])
            y0sb[(1, 0)] = emit_drain_y0(1, 0, emit_conv(0, 1, 0, xo0))
            emit_apply(0, 1, y0sb[(0, 1)])
            y0sb[(1, 1)] = emit_drain_y0(1, 1, emit_conv(0, 1, 1, xo0))
            emit_apply(1, 0, y0sb[(1, 0)])
            emit_apply(1, 1, y0sb[(1, 1)])

    nc.compile()
    _BUILD_CACHE["nc"] = nc
    return nc


def _host_prep(x, conv_w, bn_gamma, bn_beta, bn_mean, bn_var, fc1_w, fc1_b,
               fc2_w, fc2_b):
    x = np.asarray(x, np.float32)
    conv_w = np.asarray(conv_w, np.float32)
    x_ext = np.zeros((B, C, 60, 57), np.float32)
    x_ext[:, :, 5:53, 5:53] = x

    gprime = (np.asarray(bn_gamma) / np.sqrt(np.asarray(bn_var) + BN_EPS)) \
        .astype(np.float32)
    bprime = (np.asarray(bn_beta) - np.asarray(bn_mean) * gprime) \
        .astype(np.float32)
    bias_act = bprime                                    # [4, 256]

    # wt[ci, cc, oc, tap, co] = conv_w[oc*128+co, cc*128+ci, di, dj]
    w6 = conv_w.reshape(2, 128, 2, 128, 9)               # [oc, co, cc, ci, tap]
    wt = np.ascontiguousarray(w6.transpose(3, 2, 0, 4, 1))

    bias_t = np.ascontiguousarray(
        bias_act.reshape(4, 2, 128).transpose(2, 1, 0))  # [co, oc, m]
    gpr_t = np.ascontiguousarray(
        gprime.reshape(4, 2, 128).transpose(2, 1, 0))
    w1t = np.ascontiguousarray(
        np.asarray(fc1_w, np.float32).T.reshape(2, 128, 32).transpose(1, 0, 2))
    b1 = np.asarray(fc1_b, np.float32).reshape(32, 1).copy()
    w2t = np.ascontiguousarray(
        np.asarray(fc2_w, np.float32).reshape(4, 2, 128, 32)
        .transpose(3, 0, 1, 2))
    b2t = np.ascontiguousarray(
        np.asarray(fc2_b, np.float32).reshape(4, 2, 128).transpose(2, 1, 0))

    shared = dict(wt=wt, bias_act=bias_t, gpr=gpr_t, w1t=w1t, b1=b1, w2t=w2t,
                  b2t=b2t)
    in_maps = []
    for k in range(NCORES):
        i0 = I0S[k]
        slab = x_ext[:, :, i0:i0 + 16, :]                # [b, C, 16, 57]
        xs = np.ascontiguousarray(
            slab.reshape(B, 2, 128, 16, 57).transpose(2, 0, 1, 3, 4)
            .reshape(128, 4, 16, 57))
        m = dict(shared)
        m["xs"] = xs
        in_maps.append(m)
    return in_maps


def kernel(x, conv_w, bn_gamma, bn_beta, bn_mean, bn_var, fc1_w, fc1_b,
           fc2_w, fc2_b):
    from concourse import bass_utils

    nc = _build_program()
    in_maps = _host_prep(x, conv_w, bn_gamma, bn_beta, bn_mean, bn_var,
                         fc1_w, fc1_b, fc2_w, fc2_b)
    res = bass_utils.run_bass_kernel_spmd(nc, in_maps,
                                          core_ids=list(range(NCORES)))
    full = np.zeros((B, C, HOUT, HOUT), np.float32)
    for k in range(NCORES):
        o = res.results[k]["out"]                         # [128, 4, 18, 142]
        o = o.reshape(128, B, 2, NROWS, HOUT).transpose(1, 2, 0, 3, 4) \
             .reshape(B, C, NROWS, HOUT)
        if k < 7:
            full[:, :, 18 * k:18 * k + 18, :] = o
        else:
            full[:, :, 126:142, :] = o[:, :, 0:16, :]
    return full
